# revision 41
# baseline (speedup 1.0000x reference)
"""Trainium2 Bass kernel for the DagnabbitAutoEncoder DAG scan.

Strategy: level-scheduled scan (node depth = 1 + max parent depth; ~28
levels), type-sharded across the 8 NeuronCores (type t -> cores 2t, 2t+1),
fp16 datapath with fp32 PSUM accumulation.

v2 improvements over the AllGather-per-level baseline:
  * parent-swap: every node has >=1 parent in the immediately previous
    level.  Nodes whose OTHER parent is old get their parent slots swapped
    (using a row-permuted copy of W1) so gather slot 0 always reads an old
    row.  All slot-0 gather columns + their PE transposes are then
    independent of the previous AllGather and run in its shadow.
  * read-packing: rows never referenced by any later node are placed after
    the AllGather span and are not exchanged at all; together with 32-row
    padding this cuts AllGather bytes ~40%.
  * the second MLP layer runs with lhsT = h (feature-major hidden) and
    rhs = W2, producing the node-major output tile directly in PSUM - no
    output PE transposes, and the PSUM->SBUF eviction fuses the b2 add.
  * gather columns are grouped [slot0 | slot1] per level; slot-0 columns
    (old parents) are gathered and PE-transposed in the AllGather's shadow.
  * compute is phase-split at the AllGather span: unread-row tiles and the
    next level's early gathers/transposes execute inside the AllGather's
    latency; shard_out streaming rides the scalar HWDGE queue so only the
    collective-staging DMAs gate the AllGather doorbell.  The last level
    (no future readers) skips its AllGather entirely.

The host merges the 8 partial outputs (each core owns its shard rows).
"""

import math
import os

import numpy as np

R = 256
D = 256
NCORES = 8
P = 128
PSUM_N = 512
AGPAD = 32      # AllGather span padding granularity (rows)
GFUSE = int(os.environ.get("DAG_GFUSE", "0"))  # fuse adjacent gather columns
SWAP_MIN = int(os.environ.get("DAG_SWAP_MIN", "2048"))


# ---------------------------------------------------------------------------
# host-side preprocessing
# ---------------------------------------------------------------------------

def _compute_levels(idx):
    n = idx.shape[0]
    depth = np.zeros(R + n, np.int32)
    ia = idx[:, 0]
    ib = idx[:, 1]
    d = depth
    for i in range(n):
        da = d[ia[i]]
        db = d[ib[i]]
        d[R + i] = (da if da > db else db) + 1
    return depth[R:]


def _plan(idx, types):
    n = idx.shape[0]
    lv = _compute_levels(idx)
    L = int(lv.max()) if n else 0
    rowlv = np.zeros(R + n, np.int64)
    rowlv[R:] = lv
    read = np.zeros(R + n, bool)
    read[idx.ravel()] = True

    order = np.argsort(lv, kind="stable")
    lv_sorted = lv[order]
    level_nodes = []
    lo = 0
    for l in range(1, L + 1):
        hi = lo + np.searchsorted(lv_sorted[lo:], l + 1)
        level_nodes.append(order[lo:hi])
        lo = hi

    pos = np.zeros(R + n, np.int64)
    pos[:R] = np.arange(R)
    swap = np.zeros(n, bool)

    blk = R
    blk_starts = []
    specs = []       # per level: k, npad, agb, ranges
    shards = []      # per level: per-core (node_ids, slots)
    for l0 in range(L):
        l = l0 + 1
        nodes = level_nodes[l0]
        blk_starts.append(blk)
        per_core = []
        for t in range(4):
            nt = nodes[types[nodes] == t]
            per_core.append(nt[0::2])
            per_core.append(nt[1::2])
        # the parent-swap trick pays for its extra weight ranges only on
        # levels with substantial per-core shards
        use_swap = len(nodes) > SWAP_MIN
        chunks = []   # per core: (Ar, Cr, Br, Bu, Au, Cu) node-id arrays
        for c in range(NCORES):
            s = per_core[c]
            if l > 1:
                p0new = rowlv[idx[s, 0]] == l - 1
                p1new = rowlv[idx[s, 1]] == l - 1
            else:
                p0new = np.zeros(len(s), bool)
                p1new = np.zeros(len(s), bool)
            if use_swap:
                bmask = p0new & ~p1new      # swap: slot0 <- parent1 (old)
                cmask = p0new & p1new       # both parents new
            else:
                bmask = np.zeros(len(s), bool)
                cmask = p0new
            apure = ~p0new
            rd = read[R + s]
            swap[s[bmask]] = True
            if use_swap:
                # unread rows all use the swapped-weight block too, keeping
                # the B span contiguous to the end: 2 ranges instead of 3
                swap[s[~rd]] = True
            chunks.append((s[apure & rd], s[cmask & rd], s[bmask & rd],
                           s[bmask & ~rd], s[apure & ~rd], s[cmask & ~rd]))
        # uniform per-chunk start offsets so C nodes (new slot-0 parent) sit
        # at the same slots on every core and spoil at most one column
        starts = []
        off = 0
        for ci in range(6):
            w = max(len(ch[ci]) for ch in chunks)
            starts.append(off)
            off += w
            if ci == 2:                      # after Br: round AG span
                agb = (off + AGPAD - 1) // AGPAD * AGPAD
                off = agb
        cmid = starts[2]                     # A-weights span: Ar+Cr
        bend = starts[4]                     # B-weights span: Br+Bu
        npad = max(AGPAD, (off + AGPAD - 1) // AGPAD * AGPAD)
        k = (npad + P - 1) // P
        if use_swap and npad > cmid:
            ranges = []
            if cmid > 0:
                ranges.append((0, 0, cmid))
            ranges.append((1, cmid, npad))
        else:
            ranges = [(0, 0, npad)]
        core_slots = []
        for c in range(NCORES):
            ids = np.concatenate(chunks[c])
            sl = np.concatenate([st + np.arange(len(ch))
                                 for st, ch in zip(starts, chunks[c])]
                                ) if len(ids) else np.zeros(0, np.int64)
            core_slots.append((ids, sl.astype(np.int64)))
            pos[R + ids[read[R + ids]]] = blk + c * agb + sl[read[R + ids]]
        specs.append({"k": k, "npad": npad, "agb": agb, "ranges": ranges})
        shards.append(core_slots)
        blk += NCORES * agb

    K = sum(s["k"] for s in specs)
    # gidx layout per level: [slot0 cols (k) | slot1 cols (k)]
    gidx = np.zeros((NCORES, P, 2 * K), np.int32)
    src_rows = [[] for _ in range(NCORES)]
    dst_rows = [[] for _ in range(NCORES)]
    goff = 0
    soff = 0
    for l0 in range(L):
        spec = specs[l0]
        k = spec["k"]
        colmax = np.zeros(2 * k, np.int64)
        for c in range(NCORES):
            ids, sl = shards[l0][c]
            if len(ids):
                sw = swap[ids]
                pa = np.where(sw, idx[ids, 1], idx[ids, 0])   # slot0 parent
                pb = np.where(sw, idx[ids, 0], idx[ids, 1])   # slot1 parent
                pp = sl % P
                jj = sl // P
                gidx[c, pp, goff + jj] = pos[pa]
                gidx[c, pp, goff + k + jj] = pos[pb]
                np.maximum.at(colmax, jj, pos[pa])
                np.maximum.at(colmax, k + jj, pos[pb])
                src_rows[c].append(soff + sl)
                dst_rows[c].append(R + ids)
        prev_blk = blk_starts[l0 - 1] if l0 > 0 else R
        spec["early"] = [bool(colmax[col] < prev_blk) for col in range(2 * k)]
        spec["early_bound"] = int(prev_blk)
        goff += 2 * k
        soff += spec["npad"]
    src_rows = [np.concatenate(o) if o else np.zeros(0, np.int64) for o in src_rows]
    dst_rows = [np.concatenate(o) if o else np.zeros(0, np.int64) for o in dst_rows]
    return {
        "specs": specs,
        "slots": blk,
        "K": K,
        "sum_npad": soff,
        "gidx": gidx,
        "src_rows": src_rows,
        "dst_rows": dst_rows,
    }


# ---------------------------------------------------------------------------
# Bass program
# ---------------------------------------------------------------------------

def _build_program(specs, slots, K, sum_npad):
    import concourse.bass as bass
    import concourse.tile as tile
    from concourse import bacc, mybir
    from concourse.masks import make_identity

    F16 = mybir.dt.float16
    F32 = mybir.dt.float32
    I32 = mybir.dt.int32
    AF = mybir.ActivationFunctionType

    nc = bacc.Bacc("TRN2", target_bir_lowering=False, debug=False,
                   num_devices=NCORES)
    # w1 blocks: 0 = own type, 1 = own type with parent halves swapped
    t_root = nc.dram_tensor("roots", [R, D], F16, kind="ExternalInput")
    t_w1 = nc.dram_tensor("w1", [P, 2 * 2048], F16, kind="ExternalInput")
    t_w2 = nc.dram_tensor("w2", [P, 1024], F16, kind="ExternalInput")
    t_b1 = nc.dram_tensor("b1", [P, 4], F32, kind="ExternalInput")
    t_b2 = nc.dram_tensor("b2", [P, D], F32, kind="ExternalInput")
    t_gidx = nc.dram_tensor("gidx", [P, 2 * K], I32, kind="ExternalInput")
    t_sout = nc.dram_tensor("shard_out", [sum_npad, D], F16,
                            kind="ExternalOutput")
    buffer = nc.dram_tensor("buffer", [slots, D], F16, kind="Internal",
                            addr_space="Shared")
    groups = [list(range(NCORES))]
    k_max = max(s["k"] for s in specs)

    with tile.TileContext(nc) as tc:
        with (
            tc.tile_pool(name="const", bufs=1) as constp,
            tc.tile_pool(name="sbuf", bufs=2) as sbufp,
            tc.tile_pool(name="psum", bufs=1, space="PSUM") as psump,
            tc.tile_pool(name="dram", bufs=2, space="DRAM") as dramp,
        ):
            ident = constp.tile([P, P], F16)
            make_identity(nc, ident[:])
            w1_sb = constp.tile([P, 2 * 2048], F16)
            nc.sync.dma_start(w1_sb[:], t_w1[:])
            w2_sb = constp.tile([P, 1024], F16)
            nc.sync.dma_start(w2_sb[:], t_w2[:])
            b1_sb = constp.tile([P, 4], F32)
            nc.sync.dma_start(b1_sb[:], t_b1[:])
            b2_sb = constp.tile([P, D], F32)
            nc.sync.dma_start(b2_sb[:], t_b2[:])
            gidx_sb = constp.tile([P, 2 * K], I32)
            nc.sync.dma_start(gidx_sb[:], t_gidx[:])

            # roots -> buffer[0:R]
            stg = sbufp.tile([P, (R // P) * D], F16, tag="stg")
            nc.sync.dma_start(
                stg[:], t_root[:].rearrange("(j p) d -> p j d", p=P))
            nc.sync.dma_start(
                buffer[0:R, :].rearrange("(j p) d -> p j d", p=P), stg[:])

            L = len(specs)
            goffs = [0] * (L + 1)
            for l in range(L):
                goffs[l + 1] = goffs[l] + 2 * specs[l]["k"]
            soffs = [0] * (L + 1)
            for l in range(L):
                soffs[l + 1] = soffs[l] + specs[l]["npad"]
            blks = [R]
            for l in range(L):
                blks.append(blks[l] + NCORES * specs[l]["agb"])

            gx_t = {}
            xt_t = {}

            def emit_gathers(l, phase):
                spec = specs[l]
                k = spec["k"]
                if l not in gx_t:
                    gx_t[l] = sbufp.tile([P, 2 * k_max * D], F16, tag="gx",
                                         name="gx")[:, : 2 * k * D]
                gx = gx_t[l]
                early = spec["early"]
                bound = spec["early_bound"]
                goff = goffs[l]
                # fuse contiguous same-phase column runs into one DMA
                col = 0
                while col < 2 * k:
                    if early[col] != (phase == "early"):
                        col += 1
                        continue
                    c0 = col
                    if GFUSE:
                        while col < 2 * k and early[col] == (phase == "early"):
                            col += 1
                    else:
                        col += 1
                    src = buffer[0:bound, :] if phase == "early" else buffer[:]
                    nc.gpsimd.indirect_dma_start(
                        out=gx[:, c0 * D: col * D], out_offset=None,
                        in_=src,
                        in_offset=bass.IndirectOffsetOnAxis(
                            ap=gidx_sb[:, goff + c0: goff + col],
                            axis=0))

            def emit_transposes(l, phase):
                spec = specs[l]
                k = spec["k"]
                gx = gx_t[l]
                early = spec["early"]
                if l not in xt_t:
                    xt_t[l] = [sbufp.tile([P, P * k_max], F16, tag=f"xt{ic}",
                                          name=f"xt{ic}")[:, : P * k]
                               for ic in range(4)]
                xt = xt_t[l]
                for j in range(k):
                    for ic in range(4):
                        # gx col for slot0 of tile j is j; slot1 is k + j
                        col = j if ic < 2 else k + j
                        if early[col] != (phase == "early"):
                            continue
                        tp = psump.tile([P, P], F16, tag="tpose", bufs=3,
                                        name="tp")
                        nc.tensor.transpose(
                            tp[:],
                            gx[:, col * D + (ic % 2) * P:
                               col * D + (ic % 2) * P + P],
                            ident[:])
                        nc.vector.tensor_copy(xt[ic][:, j * P:(j + 1) * P],
                                              tp[:])

            for l, spec in enumerate(specs):
                k = spec["k"]
                npad = spec["npad"]
                agb = spec["agb"]
                soff = soffs[l]
                blk = blks[l]
                if l == 0:
                    emit_gathers(0, "early")
                    emit_gathers(0, "late")
                    emit_transposes(0, "early")
                    emit_transposes(0, "late")
                xt = xt_t.pop(l)
                gx_t.pop(l)

                h_sb = [sbufp.tile([P, P * k_max], F16, tag=f"h{oc}",
                                   name=f"h{oc}")[:, : npad] for oc in range(4)]
                e_sb = sbufp.tile([P, k_max * D], F16, tag="e",
                                  name="e")[:, : k * D]
                cc_in = dramp.tile([agb, D], F16, tag="cc",
                                   name="cc") if agb else None
                # phase split: cols < agb128 (read rows + straddle) before the
                # AllGather; the rest fills the AllGather's latency
                agb128 = min(npad, (agb + P - 1) // P * P)

                def emit_compute(p0, p1):
                    groups = []
                    for wb, r0, r1 in spec["ranges"]:
                        r0 = max(r0, p0)
                        r1 = min(r1, p1)
                        for g0 in range(r0, r1, PSUM_N):
                            groups.append((wb, g0, min(g0 + PSUM_N, r1)))
                    jhi = (p1 + P - 1) // P
                    jnext = p0 // P

                    def emit_l2_upto(jmax):
                        # layer 2: emb = h @ W2 + b2, node-major via lhsT = h;
                        # interleaved with L1 groups so the collective-staging
                        # DMAs (and the AllGather doorbell) fire early
                        nonlocal jnext
                        for j in range(jnext, jmax):
                            w = min(P, npad - j * P)
                            ep = psump.tile([P, D], F32, tag=f"ep{j % 2}",
                                            name="ep")[:w, :]
                            for ic in range(4):
                                nc.tensor.matmul(
                                    ep, lhsT=h_sb[ic][:, j * P: j * P + w],
                                    rhs=w2_sb[:, ic * D:(ic + 1) * D],
                                    start=(ic == 0), stop=(ic == 3))
                            nc.vector.tensor_add(
                                e_sb[:w, j * D:(j + 1) * D], ep, b2_sb[:w, :])
                            # cc DMAs alone on the sync queue gate the
                            # AllGather; shard_out rides the scalar HWDGE queue
                            nc.scalar.dma_start(
                                t_sout[soff + j * P: soff + j * P + w, :],
                                e_sb[:w, j * D:(j + 1) * D])
                            wc = min(agb - j * P, w)
                            if wc > 0:
                                nc.sync.dma_start(
                                    cc_in[j * P: j * P + wc, :],
                                    e_sb[:wc, j * D:(j + 1) * D])
                        jnext = jmax

                    # layer 1: h = gelu(x @ W1 + b1), feature-major
                    for wb, g0, g1 in groups:
                        ng = g1 - g0
                        cols = slice(g0, g1)
                        for oc in range(4):
                            hp = psump.tile([P, PSUM_N], F32,
                                            tag=f"hp{oc % 2}",
                                            name="hp")[:, :ng]
                            for ic in range(4):
                                w = w1_sb[:, wb * 2048 + ic * 512 + oc * P:
                                          wb * 2048 + ic * 512 + (oc + 1) * P]
                                nc.tensor.matmul(
                                    hp, lhsT=w, rhs=xt[ic][:, cols],
                                    start=(ic == 0), stop=(ic == 3))
                            nc.scalar.activation(
                                h_sb[oc][:, cols], hp, AF.Gelu,
                                bias=b1_sb[:, oc: oc + 1])
                        emit_l2_upto(min(jhi, g1 // P))
                    emit_l2_upto(jhi)

                emit_compute(0, agb128)
                # next level's AG-independent gathers + transposes and this
                # level's unread-row compute run in the AllGather's shadow
                if l + 1 < L:
                    emit_gathers(l + 1, "early")
                    emit_transposes(l + 1, "early")
                if agb:
                    nc.gpsimd.collective_compute(
                        "AllGather", mybir.AluOpType.bypass,
                        replica_groups=groups,
                        ins=[cc_in[:]],
                        outs=[buffer[blk: blk + NCORES * agb, :]])
                if agb128 < npad:
                    emit_compute(agb128, npad)
                if l + 1 < L:
                    emit_gathers(l + 1, "late")
                    emit_transposes(l + 1, "late")
    nc.compile()
    return nc


# ---------------------------------------------------------------------------
# entry point
# ---------------------------------------------------------------------------

_CACHE = {}


def _get_program(key, *args):
    if key not in _CACHE:
        _CACHE[key] = _build_program(*args)
    return _CACHE[key]


def kernel(root_node_embeddings, enc_W1, enc_b1, enc_W2, enc_b2,
           trunk_node_inputs_indices, trunk_node_types):
    from concourse import bass_utils

    root = np.asarray(root_node_embeddings, dtype=np.float32)
    W1 = np.asarray(enc_W1, dtype=np.float32)
    W2 = np.asarray(enc_W2, dtype=np.float32)
    b1 = np.asarray(enc_b1, dtype=np.float32)
    b2 = np.asarray(enc_b2, dtype=np.float32)
    idx = np.asarray(trunk_node_inputs_indices)
    types = np.asarray(trunk_node_types)
    if types.ndim > 1:
        types = types[:, 0]
    types = types.astype(np.int64)
    idx64 = idx.astype(np.int64)
    n = idx64.shape[0]

    plan = _plan(idx64, types)
    specs = plan["specs"]
    key = (tuple((s["k"], s["npad"], s["agb"], tuple(s["ranges"]),
                  tuple(s["early"])) for s in specs), plan["slots"])
    nc = _get_program(key, specs, plan["slots"], plan["K"], plan["sum_npad"])

    def w1tab(t):
        # [128, 2*2048] fp16: block0 = W1[t], block1 = parent-swapped W1[t]
        wn = W1[t]
        ws = np.concatenate([wn[D:], wn[:D]], 0)
        blocks = [w.reshape(4, P, 4, P).transpose(1, 0, 2, 3).reshape(P, -1)
                  for w in (wn, ws)]
        return np.ascontiguousarray(np.concatenate(blocks, 1), dtype=np.float16)

    def w2tab(t):
        # [128, 4*256] fp16: block ic = W2[t][ic*128:(ic+1)*128, :]
        return np.ascontiguousarray(
            W2[t].reshape(4, P, D).transpose(1, 0, 2).reshape(P, -1),
            dtype=np.float16)

    in_maps = []
    for c in range(NCORES):
        t = c // 2
        in_maps.append({
            "roots": np.ascontiguousarray(root, dtype=np.float16),
            "w1": w1tab(t),
            "w2": w2tab(t),
            "b1": np.ascontiguousarray(b1[t].reshape(4, P).T,
                                       dtype=np.float32),
            "b2": np.ascontiguousarray(np.tile(b2[t], (P, 1)),
                                       dtype=np.float32),
            "gidx": np.ascontiguousarray(plan["gidx"][c]),
        })

    res = bass_utils.run_bass_kernel_spmd(
        nc, in_maps, core_ids=list(range(NCORES)),
        trace=bool(int(os.environ.get("DAG_KERNEL_TRACE", "0"))))
    if res.exec_time_ns is not None:
        kernel.last_exec_time_ns = res.exec_time_ns

    out = np.zeros((R + n, D), np.float32)
    out[:R] = root
    for c in range(NCORES):
        dst = plan["dst_rows"][c]
        if len(dst):
            out[dst] = res.results[c]["shard_out"][plan["src_rows"][c]].astype(
                np.float32)
    return out


kernel.last_exec_time_ns = None
